# revision 42
# baseline (speedup 1.0000x reference)
"""Trainium2 Bass kernel for nn_LlamaMLP_HalfwayGIN_MultiAggregration.

Sharding: 16 heads -> 8 cores (2 heads/core). Each core computes its two
heads' full pipeline plus the partial (contraction-sharded) down-
projection; the host sums the 8 bf16 partials in f32.

Structure (single fused pass per core):
  phase 1: h = silu(x@Wg^T)*(x@Wu^T) per s-tile; PE-transposed fp8 copies
    hT8 = 8h (d-major) and h8p (t-pair-contiguous) emitted on the fly;
    P = h@(Wq^T Wk) gram projection per finished s-window (fp8 DR).
  phase 2+3 fused, s-window outer: per (window, head) the adjacency
    stripe streams as t-pairs; scores = hT8^T@P (fp8 DR), em = exp*adj
    (scalar+vector), sum_agg = h^T@adj (bf16), att_num = h8^T@em and the
    replicated denominator = ones^T@em (fp8 DR). The GIN MLP + down-proj
    of window sw-1 is interleaved chunk-by-chunk into window sw's pair
    loop. y1 = silu(W1b@sum + [W1ac@h + W1d@attn] fp8 DR, all x512);
    gin is folded into the down-projection host-side (Wd_h @ W2).

Scale folding (host): hT8/h8p = 8h fp8; wqT carries 1024*(Wq^T Wk) fp8,
P stored as 128*P fp8 (psum/64), exp scale 1/16384; adj16 = 16*adj^T
bf16 -> em8 = 16em fp8; w1b = 32*alpha*W1b^T bf16, w1ac8 =
64*((1+eps)W1a+W1c)^T fp8, w1d8 = 64*W1d^T fp8, silu scale 1/512;
wdT carries concat_h (Wd_h @ W2)^T bf16; output bf16.
"""

import math
import os
import numpy as np
import ml_dtypes

B, S, HID, NH, INTER = 1, 2048, 1024, 16, 4096
D = 256
NCORES = 8
HPC = NH // NCORES          # 2 heads per core
LOC = HPC * D               # 512 local intermediate dims
BF16 = ml_dtypes.bfloat16
FP8 = ml_dtypes.float8_e4m3

_CACHE = {}


def _build_nc():
    import concourse.mybir as mybir
    import concourse.tile as tile
    from concourse import bacc
    from concourse.masks import make_identity
    from contextlib import ExitStack

    f32 = mybir.dt.float32
    bf16 = mybir.dt.bfloat16
    fp8 = mybir.dt.float8e4
    AF = mybir.ActivationFunctionType
    DR = mybir.MatmulPerfMode.DoubleRow

    nc = bacc.Bacc("TRN2", target_bir_lowering=False, debug=False)

    NST = S // 128            # 16 s-tiles
    NSW = S // 512            # 4 s-windows
    NTC = S // 128            # 16 t-chunks
    NPR = NTC // 2            # 8 t-pairs
    NKC = HID // 128          # 8 k-chunks

    x_d = nc.dram_tensor("xb", [NST, 128, NKC, 128], bf16, kind="ExternalInput")
    wgu_d = nc.dram_tensor("wgub", [NKC, 2, 128, LOC], bf16,
                           kind="ExternalInput")
    adj_d = nc.dram_tensor("adjb", [NSW, HPC, 128, NTC, 512], bf16,
                           kind="ExternalInput")
    wq_d = nc.dram_tensor("wqT", [HPC, D, D], fp8, kind="ExternalInput")
    w1ac_d = nc.dram_tensor("w1acT", [HPC, D, D], fp8, kind="ExternalInput")
    w1b_d = nc.dram_tensor("w1bT", [HPC, D, D], bf16, kind="ExternalInput")
    w1d_d = nc.dram_tensor("w1dT", [D, D], fp8, kind="ExternalInput")
    wd_d = nc.dram_tensor("wdT", [LOC, HID], bf16, kind="ExternalInput")
    out_d = nc.dram_tensor("out", [S, HID], bf16, kind="ExternalOutput")

    with ExitStack() as es:
        tc = es.enter_context(tile.TileContext(nc))

        persist = es.enter_context(tc.tile_pool(name="persist", bufs=1))
        h_all = persist.tile([128, NST, LOC], bf16, name="h_all")
        h8p = persist.tile([128, NPR, 2 * HPC, 2, 128], fp8, name="h8p")
        hT8 = persist.tile([128, 2 * HPC, S], fp8, name="hT8")
        qTs = [persist.tile([128, HPC, 2, 512], fp8, name=f"qT{w}")
               for w in range(NSW)]

        wpool = es.enter_context(tc.tile_pool(name="weights", bufs=1))
        wq_sb = wpool.tile([128, 2 * HPC, D], fp8, name="wq_sb")
        w1ac_sb = wpool.tile([128, 2 * HPC, D], fp8, name="w1ac_sb")
        w1b_sb = wpool.tile([128, 2 * HPC, D], bf16, name="w1b_sb")
        w1d_sb = wpool.tile([128, 2, D], fp8, name="w1d_sb")
        wd_sb = wpool.tile([128, LOC // 128, HID], bf16, name="wd_sb")

        misc = es.enter_context(tc.tile_pool(name="misc", bufs=1))
        id_sb = misc.tile([128, 128], bf16, name="id_sb")
        ones2 = misc.tile([128, 2, 128], fp8, name="ones2")

        make_identity(nc, id_sb)
        nc.vector.memset(ones2, 1.0)

        adjpool = es.enter_context(tc.tile_pool(name="adj", bufs=1))

        # ---- phase 1: h = silu(x@WgT)*(x@WuT); hT8, h8 side copies ----
        with tc.tile_pool(name="xpool", bufs=1) as xpool, \
             tc.tile_pool(name="ps1", bufs=1, space="PSUM") as ps1, \
             tc.tile_pool(name="hstage", bufs=3) as hstage:
            x_sb = xpool.tile([128, NST, NKC, 128], bf16, name="x_sb")
            wgu_sb = xpool.tile([128, NKC, 2, LOC], bf16, name="wgu_sb")
            # wg/wu + first-half x interleaved on sync; rest of x on
            # gpsimd; small weights and adj stripes queue behind on sync so
            # phase-1 loads get the HBM bandwidth first
            nc.sync.dma_start(x_sb[:, 0], x_d[0])
            nc.sync.dma_start(wgu_sb[:, 0:4], wgu_d[0:4].rearrange(
                "c v p o -> p c v o"))
            nc.sync.dma_start(wgu_sb[:, 4:8], wgu_d[4:8].rearrange(
                "c v p o -> p c v o"))
            nc.sync.dma_start(x_sb[:, 1:4], x_d[1:4].rearrange(
                "s p c d -> p s c d"))
            nc.sync.dma_start(x_sb[:, 4:10], x_d[4:10].rearrange(
                "s p c d -> p s c d"))
            nc.sync.dma_start(x_sb[:, 10:16], x_d[10:16].rearrange(
                "s p c d -> p s c d"))
            nc.sync.dma_start(wq_sb, wq_d.rearrange("h (c p) e -> p (h c) e", p=128))
            nc.sync.dma_start(w1ac_sb, w1ac_d.rearrange("h (c p) o -> p (h c) o", p=128))
            nc.sync.dma_start(w1b_sb, w1b_d.rearrange("h (c p) o -> p (h c) o", p=128))
            nc.sync.dma_start(w1d_sb, w1d_d.rearrange("(c p) o -> p c o", p=128))
            nc.sync.dma_start(wd_sb, wd_d.rearrange("(c p) o -> p c o", p=128))
            adj_tiles = {}
            for sw in range(NSW):
                for hd in range(HPC):
                    a = adjpool.tile([128, NTC, 512], bf16,
                                     name=f"adj{sw}_{hd}", tag="adj", bufs=4)
                    nc.sync.dma_start(a, adj_d[sw, hd])
                    adj_tiles[(sw, hd)] = a

            # PE warm-up: keep the ramp going while the first x/w chunks
            # stream in (dummy transposes of the identity tile)
            warm = ps1.tile([128, 128], bf16, name="warm", tag="tr", bufs=2)
            for _ in range(56):
                nc.tensor.transpose(warm, id_sb, id_sb)

            def do_tr(st):
                # transpose s-tile st's four d-chunks (pipelined one behind)
                tr_ps = ps1.tile([128, 2 * HPC, 128], bf16, name=f"tr{st}",
                                 tag="tr", bufs=2)
                for j in range(2 * HPC):
                    col0 = j * 128
                    nc.tensor.transpose(tr_ps[:, j, :],
                                        h_all[:, st, col0:col0 + 128], id_sb)
                stsl = slice(st * 128, (st + 1) * 128)
                nc.vector.tensor_scalar_mul(hT8[:, :, stsl], tr_ps, 8.0)

            def do_qk(sw):
                # P = h @ (Wq^T Wk): one gram-projection replaces Q and K
                ssl = slice(sw * 512, (sw + 1) * 512)
                for hd in range(HPC):
                    for et in range(2):
                        ps = ps1.tile([128, 512], f32,
                                      name=f"qk{hd}_{et}_{sw}", tag="g",
                                      bufs=2)
                        nc.tensor.matmul(
                            ps,
                            wq_sb[:, hd * 2:hd * 2 + 2, et * 128:(et + 1) * 128],
                            hT8[:, hd * 2:hd * 2 + 2, ssl],
                            start=True, stop=True, perf_mode=DR)
                        nc.vector.tensor_scalar_mul(qTs[sw][:, hd, et, :],
                                                    ps, 1.0 / 64.0)

            for st in range(NST):
                g_ps = ps1.tile([128, LOC], f32, name=f"g{st}", tag="g", bufs=2)
                u_ps = ps1.tile([128, LOC], f32, name=f"u{st}", tag="u", bufs=2)
                for c in range(NKC):
                    lhsT = x_sb[:, st, c, :]
                    nc.tensor.matmul(g_ps, lhsT, wgu_sb[:, c, 0, :],
                                     start=(c == 0), stop=(c == NKC - 1))
                    nc.tensor.matmul(u_ps, lhsT, wgu_sb[:, c, 1, :],
                                     start=(c == 0), stop=(c == NKC - 1))
                if st >= 1:
                    do_tr(st - 1)
                sg = hstage.tile([128, LOC], bf16, name=f"sg{st}", tag="sg")
                nc.scalar.activation(sg, g_ps, AF.Silu)
                nc.vector.tensor_mul(h_all[:, st, :], sg, u_ps)
                nc.vector.tensor_scalar_mul(h8p[:, st // 2, :, st % 2, :],
                                            h_all[:, st, :], 8.0)
                if st % 4 == 3 and st >= 7:
                    do_qk(st // 4 - 1)
            do_tr(NST - 1)
            do_qk(NSW - 1)
            # pre-load the Exp activation table so sw0's first exp doesn't
            # pay the Silu->Exp table switch on the critical path
            exwarm = hstage.tile([1, 128], bf16, name="exwarm", tag="sg")
            nc.scalar.activation(exwarm, id_sb[0:1, :], AF.Exp)

        # ---- phase 2+3 fused, sw-outer; p3 of window sw-1 interleaved
        # into window sw's attention pair loop ----
        with tc.tile_pool(name="stream", bufs=1) as strm, \
             tc.tile_pool(name="outp", bufs=2) as outp, \
             tc.tile_pool(name="ps2", bufs=1, space="PSUM") as ps2:

            def make_p3(sw, hd_res):
                """Phase-3 chunk closures for window sw (16 chunks)."""
                ssl = slice(sw * 512, (sw + 1) * 512)
                y1Ts = [strm.tile([128, 2, 512], bf16, name=f"y1{sw}_{hd}",
                                  tag=f"y1_{hd}", bufs=2) for hd in range(HPC)]
                chunks = []

                def y1_chunk(hd, ot):
                    sumT, attnT = hd_res[hd]
                    osl = slice(ot * 128, (ot + 1) * 128)
                    y1_ps = ps2.tile([128, 512], f32,
                                     name=f"y1p{sw}_{hd}_{ot}", tag="mm",
                                     bufs=3)
                    for dc in range(2):
                        nc.tensor.matmul(y1_ps, w1b_sb[:, hd * 2 + dc, osl],
                                         sumT[:, dc, :],
                                         start=(dc == 0), stop=False)
                    nc.tensor.matmul(y1_ps, w1ac_sb[:, hd * 2:hd * 2 + 2, osl],
                                     hT8[:, hd * 2:hd * 2 + 2, ssl],
                                     start=False, stop=False, perf_mode=DR)
                    nc.tensor.matmul(y1_ps, w1d_sb[:, :, osl], attnT,
                                     start=False, stop=True, perf_mode=DR)
                    nc.scalar.activation(y1Ts[hd][:, ot, :], y1_ps, AF.Silu,
                                         scale=1.0 / 512.0)

                o_sbs = {}

                def down_chunk(r, nw):
                    st = sw * 4 + r
                    rsl = slice(r * 128, (r + 1) * 128)
                    if nw == 0:
                        o_sbs[r] = outp.tile([128, HID], bf16, name=f"o{st}",
                                             tag="o")
                    o_sb = o_sbs[r]
                    d_ps = ps2.tile([128, 512], f32, name=f"d{st}_{nw}",
                                    tag="mm", bufs=3)
                    for j in range(LOC // 128):
                        nc.tensor.matmul(d_ps, y1Ts[j // 2][:, j % 2, rsl],
                                         wd_sb[:, j, nw * 512:(nw + 1) * 512],
                                         start=(j == 0),
                                         stop=(j == LOC // 128 - 1))
                    if nw == 0:
                        nc.vector.tensor_copy(o_sb[:, 0:512], d_ps)
                    else:
                        nc.vector.tensor_copy(o_sb[:, 512:1024], d_ps)
                        stsl = slice(st * 128, (st + 1) * 128)
                        nc.sync.dma_start(out_d[stsl, :], o_sb)

                def pair(f, a, b):
                    def g():
                        f(*a)
                        f(*b)
                    return g
                chunks.append(pair(y1_chunk, (0, 0), (0, 1)))
                chunks.append(pair(y1_chunk, (1, 0), (1, 1)))
                for r in range(4):
                    for nw in range(2):
                        chunks.append(lambda r=r, nw=nw: down_chunk(r, nw))
                return chunks

            pending = []
            for sw in range(NSW):
                ssl = slice(sw * 512, (sw + 1) * 512)
                hd_res = []
                for hd in range(HPC):
                    adj_sb = adj_tiles[(sw, hd)]
                    sum_ps = ps2.tile([128, 2, 512], f32,
                                      name=f"sum{sw}_{hd}", tag="sum")
                    att_ps = ps2.tile([128, 2, 512], f32,
                                      name=f"att{sw}_{hd}", tag="att")
                    den_ps = ps2.tile([128, 512], f32,
                                      name=f"den{sw}_{hd}", tag="den", bufs=1)
                    em_tiles = {}
                    for pr in range(NPR + 1):
                        # consume pair pr-1 first so PE has ready work queued
                        # ahead of the (possibly psum-bank-waiting) scores
                        if pr >= 1:
                            p = pr - 1
                            em_p = em_tiles.pop(p)
                            first, last = p == 0, p == NPR - 1
                            for dc in range(2):
                                c0 = hd * D + dc * 128
                                for i in range(2):
                                    t = 2 * p + i
                                    nc.tensor.matmul(
                                        sum_ps[:, dc, :],
                                        h_all[:, t, c0:c0 + 128],
                                        adj_sb[:, t, :],
                                        start=(first and i == 0),
                                        stop=(last and i == 1))
                            nc.tensor.matmul(den_ps, ones2, em_p,
                                             start=first, stop=last,
                                             perf_mode=DR)
                            for dc in range(2):
                                nc.tensor.matmul(
                                    att_ps[:, dc, :],
                                    h8p[:, p, hd * 2 + dc, :, :],
                                    em_p, start=first, stop=last,
                                    perf_mode=DR)
                            if pending:
                                ck = pending.pop(0)
                                if ck is not None:
                                    ck()
                        if pr < NPR:
                            em8 = strm.tile([128, 2, 512], fp8,
                                            name=f"em{sw}_{hd}_{pr}",
                                            tag="em", bufs=8)
                            ex = strm.tile([128, 2, 512], bf16,
                                           name=f"ex{sw}_{hd}_{pr}",
                                           tag="ex", bufs=6)
                            for i in range(2):
                                t = 2 * pr + i
                                tsl = slice(t * 128, (t + 1) * 128)
                                sc_ps = ps2.tile([128, 512], f32,
                                                 name=f"sc{sw}_{hd}_{pr}_{i}",
                                                 tag="mm", bufs=3)
                                nc.tensor.matmul(sc_ps,
                                                 hT8[:, hd * 2:hd * 2 + 2, tsl],
                                                 qTs[sw][:, hd, :, :],
                                                 start=True, stop=True,
                                                 perf_mode=DR)
                                nc.scalar.activation(ex[:, i, :], sc_ps, AF.Exp,
                                                     scale=1.0 / 16384.0)
                            nc.vector.tensor_mul(em8, ex,
                                                 adj_sb[:, 2 * pr:2 * pr + 2, :])
                            em_tiles[pr] = em8

                    # evictions: sumT copy, recip (replicated den -> direct
                    # multiplier), attnT muls; all on vector
                    sumT = strm.tile([128, 2, 512], bf16, name=f"sumT{sw}_{hd}",
                                     tag="sumT", bufs=2)
                    nc.vector.tensor_copy(sumT, sum_ps)
                    rb_sb = strm.tile([128, 512], f32, name=f"rb{sw}_{hd}",
                                      tag="rb", bufs=2)
                    nc.vector.reciprocal_approx_fast(rb_sb, den_ps)
                    attnT = strm.tile([128, 2, 512], fp8, name=f"at{sw}_{hd}",
                                      tag="attnT", bufs=2)
                    for dc in range(2):
                        nc.vector.tensor_mul(attnT[:, dc, :],
                                             att_ps[:, dc, :], rb_sb)
                    hd_res.append((sumT, attnT))
                pending.extend(make_p3(sw, hd_res))
            for c in pending:
                if c is not None:
                    c()

    nc.compile()
    return nc


def _prep_in_maps(x, adjacency, Wg, Wu, Wd, eps, alpha, Wq, Wk, W1, W2):
    f = lambda a: np.ascontiguousarray(a, dtype=np.float32)
    x, adjacency = f(x), f(adjacency)
    Wg, Wu, Wd, Wq, Wk, W1, W2 = map(f, (Wg, Wu, Wd, Wq, Wk, W1, W2))
    eps, alpha = f(eps), f(alpha)
    b16 = lambda a: np.ascontiguousarray(a).astype(BF16)
    f8 = lambda a: np.ascontiguousarray(a).astype(FP8)

    NST, NKC, NSW, NTC = S // 128, HID // 128, S // 512, S // 128
    # x blocks: [st, p, c, sl] = x[0, st*128+sl, c*128+p]
    xb = b16(x[0].reshape(NST, 128, NKC, 128).transpose(0, 3, 2, 1))

    W1a = W1[:, 0:D]
    W1b = W1[:, D:2 * D]
    W1c = W1[:, 2 * D:3 * D]
    W1d = W1[:, 3 * D:4 * D]

    in_maps = []
    for i in range(NCORES):
        hs = slice(i * HPC, (i + 1) * HPC)
        c0, c1 = i * LOC, (i + 1) * LOC
        # adj blocks: [sw, hd, p, tt, sl] = 16*adj[hd, sw*512+sl, tt*128+p]
        a = adjacency[0, hs].reshape(HPC, NSW, 512, NTC, 128)
        adjb = b16(16.0 * a.transpose(1, 0, 4, 3, 2))
        w1ac = np.stack([64.0 * ((1.0 + eps[h]) * W1a + W1c).T
                         for h in range(i * HPC, (i + 1) * HPC)])
        w1b = np.stack([32.0 * alpha[h] * W1b.T
                        for h in range(i * HPC, (i + 1) * HPC)])
        in_maps.append({
            "xb": xb,
            "wgub": b16(np.stack([Wg[c0:c1].T.reshape(NKC, 128, LOC),
                                  Wu[c0:c1].T.reshape(NKC, 128, LOC)],
                                 axis=1)),
            "adjb": adjb,
            "wqT": f8(np.stack([1024.0 * (Wq[h].T @ Wk[h])
                                for h in range(i * HPC, (i + 1) * HPC)])),
            "w1acT": f8(w1ac),
            "w1bT": b16(w1b),
            "w1dT": f8(64.0 * W1d.T),
            "wdT": b16(np.concatenate(
                [(Wd[:, (i * HPC + hd) * D:(i * HPC + hd + 1) * D] @ W2).T
                 for hd in range(HPC)], axis=0)),
        })
    return in_maps


def _run(inputs, trace=False, trace_kwargs=None):
    from concourse.bass_utils import run_bass_kernel_spmd

    if "nc" not in _CACHE:
        _CACHE["nc"] = _build_nc()
    nc = _CACHE["nc"]
    in_maps = _prep_in_maps(**inputs)
    res = run_bass_kernel_spmd(nc, in_maps, list(range(NCORES)),
                               trace=trace, **(trace_kwargs or {}))
    out = np.zeros((S, HID), np.float32)
    for r in res.results:
        out += np.asarray(r["out"], dtype=np.float32)
    return out.reshape(B, S, HID), res


def kernel(**inputs) -> np.ndarray:
    out, _ = _run(inputs, trace=False)
    return out


# revision 43
# speedup vs baseline: 1.0053x; 1.0053x over previous
"""Trainium2 Bass kernel for nn_LlamaMLP_HalfwayGIN_MultiAggregration.

Sharding: 16 heads -> 8 cores (2 heads/core). Each core computes its two
heads' full pipeline plus the partial (contraction-sharded) down-
projection; the host sums the 8 bf16 partials in f32.

Structure (single fused pass per core):
  phase 1: h = silu(x@Wg^T)*(x@Wu^T) per s-tile; PE-transposed fp8 copies
    hT8 = 8h (d-major) and h8p (t-pair-contiguous) emitted on the fly;
    P = h@(Wq^T Wk) gram projection per finished s-window (fp8 DR).
  phase 2+3 fused, s-window outer: per (window, head) the adjacency
    stripe streams as t-pairs; scores = hT8^T@P (fp8 DR), em = exp*adj
    (scalar+vector), sum_agg = h^T@adj (bf16), att_num = h8^T@em and the
    replicated denominator = ones^T@em (fp8 DR). The GIN MLP + down-proj
    of window sw-1 is interleaved chunk-by-chunk into window sw's pair
    loop. y1 = silu(W1b@sum + [W1ac@h + W1d@attn] fp8 DR, all x512);
    gin is folded into the down-projection host-side (Wd_h @ W2).

Scale folding (host): hT8/h8p = 8h fp8; wqT carries 1024*(Wq^T Wk) fp8,
P stored as 128*P fp8 (psum/64), exp scale 1/16384; adj16 = 16*adj^T
bf16 -> em8 = 16em fp8; w1b = 32*alpha*W1b^T bf16, w1ac8 =
64*((1+eps)W1a+W1c)^T fp8, w1d8 = 64*W1d^T fp8, silu scale 1/512;
wdT carries concat_h (Wd_h @ W2)^T bf16; output bf16.
"""

import math
import os
import numpy as np
import ml_dtypes

B, S, HID, NH, INTER = 1, 2048, 1024, 16, 4096
D = 256
NCORES = 8
HPC = NH // NCORES          # 2 heads per core
LOC = HPC * D               # 512 local intermediate dims
BF16 = ml_dtypes.bfloat16
FP8 = ml_dtypes.float8_e4m3

_CACHE = {}


def _build_nc():
    import concourse.mybir as mybir
    import concourse.tile as tile
    from concourse import bacc
    from concourse.masks import make_identity
    from contextlib import ExitStack

    f32 = mybir.dt.float32
    bf16 = mybir.dt.bfloat16
    fp8 = mybir.dt.float8e4
    AF = mybir.ActivationFunctionType
    DR = mybir.MatmulPerfMode.DoubleRow

    nc = bacc.Bacc("TRN2", target_bir_lowering=False, debug=False)

    NST = S // 128            # 16 s-tiles
    NSW = S // 512            # 4 s-windows
    NTC = S // 128            # 16 t-chunks
    NPR = NTC // 2            # 8 t-pairs
    NKC = HID // 128          # 8 k-chunks

    x_d = nc.dram_tensor("xb", [NST, 128, NKC, 128], bf16, kind="ExternalInput")
    wgu_d = nc.dram_tensor("wgub", [NKC, 2, 128, LOC], bf16,
                           kind="ExternalInput")
    adj_d = nc.dram_tensor("adjb", [NSW, HPC, 128, NTC, 512], bf16,
                           kind="ExternalInput")
    wq_d = nc.dram_tensor("wqT", [HPC, D, D], fp8, kind="ExternalInput")
    w1ac_d = nc.dram_tensor("w1acT", [HPC, D, D], fp8, kind="ExternalInput")
    w1b_d = nc.dram_tensor("w1bT", [HPC, D, D], bf16, kind="ExternalInput")
    w1d_d = nc.dram_tensor("w1dT", [D, D], fp8, kind="ExternalInput")
    wd_d = nc.dram_tensor("wdT", [LOC, HID], bf16, kind="ExternalInput")
    out_d = nc.dram_tensor("out", [S, HID], bf16, kind="ExternalOutput")

    with ExitStack() as es:
        tc = es.enter_context(tile.TileContext(nc))

        persist = es.enter_context(tc.tile_pool(name="persist", bufs=1))
        h_all = persist.tile([128, NST, LOC], bf16, name="h_all")
        h8p = persist.tile([128, NPR, 2 * HPC, 2, 128], fp8, name="h8p")
        hT8 = persist.tile([128, 2 * HPC, S], fp8, name="hT8")
        qTs = [persist.tile([128, HPC, 2, 512], fp8, name=f"qT{w}")
               for w in range(NSW)]

        wpool = es.enter_context(tc.tile_pool(name="weights", bufs=1))
        wq_sb = wpool.tile([128, 2 * HPC, D], fp8, name="wq_sb")
        w1ac_sb = wpool.tile([128, 2 * HPC, D], fp8, name="w1ac_sb")
        w1b_sb = wpool.tile([128, 2 * HPC, D], bf16, name="w1b_sb")
        w1d_sb = wpool.tile([128, 2, D], fp8, name="w1d_sb")
        wd_sb = wpool.tile([128, LOC // 128, HID], bf16, name="wd_sb")

        misc = es.enter_context(tc.tile_pool(name="misc", bufs=1))
        id_sb = misc.tile([128, 128], bf16, name="id_sb")
        ones2 = misc.tile([128, 2, 128], fp8, name="ones2")

        make_identity(nc, id_sb)
        nc.vector.memset(ones2, 1.0)

        adjpool = es.enter_context(tc.tile_pool(name="adj", bufs=1))

        # ---- phase 1: h = silu(x@WgT)*(x@WuT); hT8, h8 side copies ----
        with tc.tile_pool(name="xpool", bufs=1) as xpool, \
             tc.tile_pool(name="ps1", bufs=1, space="PSUM") as ps1, \
             tc.tile_pool(name="hstage", bufs=3) as hstage:
            x_sb = xpool.tile([128, NST, NKC, 128], bf16, name="x_sb")
            wgu_sb = xpool.tile([128, NKC, 2, LOC], bf16, name="wgu_sb")
            # wg/wu + first-half x interleaved on sync; rest of x on
            # gpsimd; small weights and adj stripes queue behind on sync so
            # phase-1 loads get the HBM bandwidth first
            nc.sync.dma_start(x_sb[:, 0], x_d[0])
            nc.sync.dma_start(wgu_sb[:, 0:4], wgu_d[0:4].rearrange(
                "c v p o -> p c v o"))
            nc.sync.dma_start(x_sb[:, 1:4], x_d[1:4].rearrange(
                "s p c d -> p s c d"))
            nc.sync.dma_start(wgu_sb[:, 4:8], wgu_d[4:8].rearrange(
                "c v p o -> p c v o"))
            nc.sync.dma_start(x_sb[:, 4:10], x_d[4:10].rearrange(
                "s p c d -> p s c d"))
            nc.sync.dma_start(x_sb[:, 10:16], x_d[10:16].rearrange(
                "s p c d -> p s c d"))
            nc.sync.dma_start(wq_sb, wq_d.rearrange("h (c p) e -> p (h c) e", p=128))
            nc.sync.dma_start(w1ac_sb, w1ac_d.rearrange("h (c p) o -> p (h c) o", p=128))
            nc.sync.dma_start(w1b_sb, w1b_d.rearrange("h (c p) o -> p (h c) o", p=128))
            nc.sync.dma_start(w1d_sb, w1d_d.rearrange("(c p) o -> p c o", p=128))
            nc.sync.dma_start(wd_sb, wd_d.rearrange("(c p) o -> p c o", p=128))
            adj_tiles = {}
            for sw in range(NSW):
                for hd in range(HPC):
                    a = adjpool.tile([128, NTC, 512], bf16,
                                     name=f"adj{sw}_{hd}", tag="adj", bufs=4)
                    nc.sync.dma_start(a, adj_d[sw, hd])
                    adj_tiles[(sw, hd)] = a

            # PE warm-up: keep the ramp going while the first x/w chunks
            # stream in (dummy transposes of the identity tile)
            warm = ps1.tile([128, 128], bf16, name="warm", tag="tr", bufs=2)
            for _ in range(56):
                nc.tensor.transpose(warm, id_sb, id_sb)

            def do_tr(st):
                # transpose s-tile st's four d-chunks (pipelined one behind)
                tr_ps = ps1.tile([128, 2 * HPC, 128], bf16, name=f"tr{st}",
                                 tag="tr", bufs=2)
                for j in range(2 * HPC):
                    col0 = j * 128
                    nc.tensor.transpose(tr_ps[:, j, :],
                                        h_all[:, st, col0:col0 + 128], id_sb)
                stsl = slice(st * 128, (st + 1) * 128)
                nc.vector.tensor_scalar_mul(hT8[:, :, stsl], tr_ps, 8.0)

            def do_qk(sw):
                # P = h @ (Wq^T Wk): one gram-projection replaces Q and K
                ssl = slice(sw * 512, (sw + 1) * 512)
                for hd in range(HPC):
                    for et in range(2):
                        ps = ps1.tile([128, 512], f32,
                                      name=f"qk{hd}_{et}_{sw}", tag="g",
                                      bufs=2)
                        nc.tensor.matmul(
                            ps,
                            wq_sb[:, hd * 2:hd * 2 + 2, et * 128:(et + 1) * 128],
                            hT8[:, hd * 2:hd * 2 + 2, ssl],
                            start=True, stop=True, perf_mode=DR)
                        nc.vector.tensor_scalar_mul(qTs[sw][:, hd, et, :],
                                                    ps, 1.0 / 64.0)

            gus = {}

            def gu_mm(st, cs, ce):
                g_ps, u_ps = gus[st]
                for c in range(cs, ce):
                    lhsT = x_sb[:, st, c, :]
                    nc.tensor.matmul(g_ps, lhsT, wgu_sb[:, c, 0, :],
                                     start=(c == 0), stop=(c == NKC - 1))
                    nc.tensor.matmul(u_ps, lhsT, wgu_sb[:, c, 1, :],
                                     start=(c == 0), stop=(c == NKC - 1))

            def finish(st):
                g_ps, u_ps = gus.pop(st)
                if st >= 1:
                    do_tr(st - 1)
                sg = hstage.tile([128, LOC], bf16, name=f"sg{st}", tag="sg")
                nc.scalar.activation(sg, g_ps, AF.Silu)
                nc.vector.tensor_mul(h_all[:, st, :], sg, u_ps)
                nc.vector.tensor_scalar_mul(h8p[:, st // 2, :, st % 2, :],
                                            h_all[:, st, :], 8.0)
                if st % 4 == 3 and st >= 7:
                    do_qk(st // 4 - 1)

            # st0/st1 with split contraction: both first halves run while
            # the second wgu half is still in flight
            for st in (0, 1):
                gus[st] = (ps1.tile([128, LOC], f32, name=f"g{st}", tag="g",
                                    bufs=2),
                           ps1.tile([128, LOC], f32, name=f"u{st}", tag="u",
                                    bufs=2))
            gu_mm(0, 0, 4)
            gu_mm(1, 0, 4)
            gu_mm(0, 4, 8)
            gu_mm(1, 4, 8)
            finish(0)
            finish(1)
            for st in range(2, NST):
                gus[st] = (ps1.tile([128, LOC], f32, name=f"g{st}", tag="g",
                                    bufs=2),
                           ps1.tile([128, LOC], f32, name=f"u{st}", tag="u",
                                    bufs=2))
                gu_mm(st, 0, NKC)
                finish(st)
            do_tr(NST - 1)
            do_qk(NSW - 1)
            # pre-load the Exp activation table so sw0's first exp doesn't
            # pay the Silu->Exp table switch on the critical path
            exwarm = hstage.tile([1, 128], bf16, name="exwarm", tag="sg")
            nc.scalar.activation(exwarm, id_sb[0:1, :], AF.Exp)

        # ---- phase 2+3 fused, sw-outer; p3 of window sw-1 interleaved
        # into window sw's attention pair loop ----
        with tc.tile_pool(name="stream", bufs=1) as strm, \
             tc.tile_pool(name="outp", bufs=2) as outp, \
             tc.tile_pool(name="ps2", bufs=1, space="PSUM") as ps2:

            def make_p3(sw, hd_res):
                """Phase-3 chunk closures for window sw (16 chunks)."""
                ssl = slice(sw * 512, (sw + 1) * 512)
                y1Ts = [strm.tile([128, 2, 512], bf16, name=f"y1{sw}_{hd}",
                                  tag=f"y1_{hd}", bufs=2) for hd in range(HPC)]
                chunks = []

                def y1_chunk(hd, ot):
                    sumT, attnT = hd_res[hd]
                    osl = slice(ot * 128, (ot + 1) * 128)
                    y1_ps = ps2.tile([128, 512], f32,
                                     name=f"y1p{sw}_{hd}_{ot}", tag="mm",
                                     bufs=3)
                    for dc in range(2):
                        nc.tensor.matmul(y1_ps, w1b_sb[:, hd * 2 + dc, osl],
                                         sumT[:, dc, :],
                                         start=(dc == 0), stop=False)
                    nc.tensor.matmul(y1_ps, w1ac_sb[:, hd * 2:hd * 2 + 2, osl],
                                     hT8[:, hd * 2:hd * 2 + 2, ssl],
                                     start=False, stop=False, perf_mode=DR)
                    nc.tensor.matmul(y1_ps, w1d_sb[:, :, osl], attnT,
                                     start=False, stop=True, perf_mode=DR)
                    nc.scalar.activation(y1Ts[hd][:, ot, :], y1_ps, AF.Silu,
                                         scale=1.0 / 512.0)

                o_sbs = {}

                def down_chunk(r, nw):
                    st = sw * 4 + r
                    rsl = slice(r * 128, (r + 1) * 128)
                    if nw == 0:
                        o_sbs[r] = outp.tile([128, HID], bf16, name=f"o{st}",
                                             tag="o")
                    o_sb = o_sbs[r]
                    d_ps = ps2.tile([128, 512], f32, name=f"d{st}_{nw}",
                                    tag="mm", bufs=3)
                    for j in range(LOC // 128):
                        nc.tensor.matmul(d_ps, y1Ts[j // 2][:, j % 2, rsl],
                                         wd_sb[:, j, nw * 512:(nw + 1) * 512],
                                         start=(j == 0),
                                         stop=(j == LOC // 128 - 1))
                    if nw == 0:
                        nc.vector.tensor_copy(o_sb[:, 0:512], d_ps)
                    else:
                        nc.vector.tensor_copy(o_sb[:, 512:1024], d_ps)
                        stsl = slice(st * 128, (st + 1) * 128)
                        nc.sync.dma_start(out_d[stsl, :], o_sb)

                def pair(f, a, b):
                    def g():
                        f(*a)
                        f(*b)
                    return g
                chunks.append(pair(y1_chunk, (0, 0), (0, 1)))
                chunks.append(pair(y1_chunk, (1, 0), (1, 1)))
                for r in range(4):
                    for nw in range(2):
                        chunks.append(lambda r=r, nw=nw: down_chunk(r, nw))
                return chunks

            pending = []
            for sw in range(NSW):
                ssl = slice(sw * 512, (sw + 1) * 512)
                hd_res = []
                for hd in range(HPC):
                    adj_sb = adj_tiles[(sw, hd)]
                    sum_ps = ps2.tile([128, 2, 512], f32,
                                      name=f"sum{sw}_{hd}", tag="sum")
                    att_ps = ps2.tile([128, 2, 512], f32,
                                      name=f"att{sw}_{hd}", tag="att")
                    den_ps = ps2.tile([128, 512], f32,
                                      name=f"den{sw}_{hd}", tag="den", bufs=1)
                    em_tiles = {}
                    for pr in range(NPR + 1):
                        # consume pair pr-1 first so PE has ready work queued
                        # ahead of the (possibly psum-bank-waiting) scores
                        if pr >= 1:
                            p = pr - 1
                            em_p = em_tiles.pop(p)
                            first, last = p == 0, p == NPR - 1
                            for dc in range(2):
                                c0 = hd * D + dc * 128
                                for i in range(2):
                                    t = 2 * p + i
                                    nc.tensor.matmul(
                                        sum_ps[:, dc, :],
                                        h_all[:, t, c0:c0 + 128],
                                        adj_sb[:, t, :],
                                        start=(first and i == 0),
                                        stop=(last and i == 1))
                            nc.tensor.matmul(den_ps, ones2, em_p,
                                             start=first, stop=last,
                                             perf_mode=DR)
                            for dc in range(2):
                                nc.tensor.matmul(
                                    att_ps[:, dc, :],
                                    h8p[:, p, hd * 2 + dc, :, :],
                                    em_p, start=first, stop=last,
                                    perf_mode=DR)
                            if pending:
                                ck = pending.pop(0)
                                if ck is not None:
                                    ck()
                        if pr < NPR:
                            em8 = strm.tile([128, 2, 512], fp8,
                                            name=f"em{sw}_{hd}_{pr}",
                                            tag="em", bufs=8)
                            ex = strm.tile([128, 2, 512], bf16,
                                           name=f"ex{sw}_{hd}_{pr}",
                                           tag="ex", bufs=6)
                            for i in range(2):
                                t = 2 * pr + i
                                tsl = slice(t * 128, (t + 1) * 128)
                                sc_ps = ps2.tile([128, 512], f32,
                                                 name=f"sc{sw}_{hd}_{pr}_{i}",
                                                 tag="mm", bufs=3)
                                nc.tensor.matmul(sc_ps,
                                                 hT8[:, hd * 2:hd * 2 + 2, tsl],
                                                 qTs[sw][:, hd, :, :],
                                                 start=True, stop=True,
                                                 perf_mode=DR)
                                nc.scalar.activation(ex[:, i, :], sc_ps, AF.Exp,
                                                     scale=1.0 / 16384.0)
                            nc.vector.tensor_mul(em8, ex,
                                                 adj_sb[:, 2 * pr:2 * pr + 2, :])
                            em_tiles[pr] = em8

                    # evictions: sumT copy, recip (replicated den -> direct
                    # multiplier), attnT muls; all on vector
                    sumT = strm.tile([128, 2, 512], bf16, name=f"sumT{sw}_{hd}",
                                     tag="sumT", bufs=2)
                    nc.vector.tensor_copy(sumT, sum_ps)
                    rb_sb = strm.tile([128, 512], f32, name=f"rb{sw}_{hd}",
                                      tag="rb", bufs=2)
                    nc.vector.reciprocal_approx_fast(rb_sb, den_ps)
                    attnT = strm.tile([128, 2, 512], fp8, name=f"at{sw}_{hd}",
                                      tag="attnT", bufs=2)
                    for dc in range(2):
                        nc.vector.tensor_mul(attnT[:, dc, :],
                                             att_ps[:, dc, :], rb_sb)
                    hd_res.append((sumT, attnT))
                pending.extend(make_p3(sw, hd_res))
            for c in pending:
                if c is not None:
                    c()

    nc.compile()
    return nc


def _prep_in_maps(x, adjacency, Wg, Wu, Wd, eps, alpha, Wq, Wk, W1, W2):
    f = lambda a: np.ascontiguousarray(a, dtype=np.float32)
    x, adjacency = f(x), f(adjacency)
    Wg, Wu, Wd, Wq, Wk, W1, W2 = map(f, (Wg, Wu, Wd, Wq, Wk, W1, W2))
    eps, alpha = f(eps), f(alpha)
    b16 = lambda a: np.ascontiguousarray(a).astype(BF16)
    f8 = lambda a: np.ascontiguousarray(a).astype(FP8)

    NST, NKC, NSW, NTC = S // 128, HID // 128, S // 512, S // 128
    # x blocks: [st, p, c, sl] = x[0, st*128+sl, c*128+p]
    xb = b16(x[0].reshape(NST, 128, NKC, 128).transpose(0, 3, 2, 1))

    W1a = W1[:, 0:D]
    W1b = W1[:, D:2 * D]
    W1c = W1[:, 2 * D:3 * D]
    W1d = W1[:, 3 * D:4 * D]

    in_maps = []
    for i in range(NCORES):
        hs = slice(i * HPC, (i + 1) * HPC)
        c0, c1 = i * LOC, (i + 1) * LOC
        # adj blocks: [sw, hd, p, tt, sl] = 16*adj[hd, sw*512+sl, tt*128+p]
        a = adjacency[0, hs].reshape(HPC, NSW, 512, NTC, 128)
        adjb = b16(16.0 * a.transpose(1, 0, 4, 3, 2))
        w1ac = np.stack([64.0 * ((1.0 + eps[h]) * W1a + W1c).T
                         for h in range(i * HPC, (i + 1) * HPC)])
        w1b = np.stack([32.0 * alpha[h] * W1b.T
                        for h in range(i * HPC, (i + 1) * HPC)])
        in_maps.append({
            "xb": xb,
            "wgub": b16(np.stack([Wg[c0:c1].T.reshape(NKC, 128, LOC),
                                  Wu[c0:c1].T.reshape(NKC, 128, LOC)],
                                 axis=1)),
            "adjb": adjb,
            "wqT": f8(np.stack([1024.0 * (Wq[h].T @ Wk[h])
                                for h in range(i * HPC, (i + 1) * HPC)])),
            "w1acT": f8(w1ac),
            "w1bT": b16(w1b),
            "w1dT": f8(64.0 * W1d.T),
            "wdT": b16(np.concatenate(
                [(Wd[:, (i * HPC + hd) * D:(i * HPC + hd + 1) * D] @ W2).T
                 for hd in range(HPC)], axis=0)),
        })
    return in_maps


def _run(inputs, trace=False, trace_kwargs=None):
    from concourse.bass_utils import run_bass_kernel_spmd

    if "nc" not in _CACHE:
        _CACHE["nc"] = _build_nc()
    nc = _CACHE["nc"]
    in_maps = _prep_in_maps(**inputs)
    res = run_bass_kernel_spmd(nc, in_maps, list(range(NCORES)),
                               trace=trace, **(trace_kwargs or {}))
    out = np.zeros((S, HID), np.float32)
    for r in res.results:
        out += np.asarray(r["out"], dtype=np.float32)
    return out.reshape(B, S, HID), res


def kernel(**inputs) -> np.ndarray:
    out, _ = _run(inputs, trace=False)
    return out
